# revision 12
# baseline (speedup 1.0000x reference)
"""MoE top-k routing + grouped down-proj GEMM + reduce-scatter for trn2 (8 cores).

Problem: intermediate_states [4, 2048, 1024] f16 (rank-sharded expanded-token
activations), w [4, 8, 1024, 2048] f16 (rank-sharded per-expert down-proj),
router_logits [1024, 8] f32, topk=2.  Output [4, 256, 2048] f16.

Strategy: the sparse compute is, per expanded token tk routed to expert e(tk):
y_part[tk] = gate(tk) * (x_full[tk] @ w_full[e(tk)]) with x_full [TK, 4096]
(rank dim folded into the contraction) and w_full[e] [4096, 2048].

Work is decomposed into jobs of (one 128-token tile of one expert) x (one
K-half of 2048).  Each core runs JOBS=5 such jobs: jobs 0-2 share W-slice A,
jobs 3-4 share W-slice B, where a slice is one (expert, K-half) [2048, 2048]
f16 weight block.  The host packs the (expert, K-half) groups (each of
ceil(c_e/128) jobs) into the 8 cores' A/B slots; with sum(ceil(c_e/128)) <= 23
and all c_e <= 384 this always fits, giving near-perfectly balanced PE load
(5 * 16 * 4 matmuls of [128,128]@[128,512] per core).  Each job accumulates in
fp32 PSUM over 16 K-subtiles and applies its fp32 routing gate as a
per-partition scale at PSUM eviction.  No collective is needed: the host sums
each final token's 4 partial rows (topk=2 experts x 2 K-halves).  Routing /
top-k / softmax runs on host in numpy (1024x8 logits - negligible).

Fallback: if some expert has more than 384 tokens the job packing doesn't fit;
an expert-per-core kernel (full K=4096, capacity padded to 128) is used
instead, launched as many times as needed.
"""

import numpy as np

R, T_TOK, TOPK, E = 4, 1024, 2, 8
I_PR, H = 1024, 2048
K = R * I_PR            # 4096 contraction
P = 128
NF = 512                # matmul free-dim (one PSUM bank of fp32)
NH = H // NF            # 4
N_CORES = 8

# job mode
KH = K // 2             # 2048 per K-half
KS2 = KH // P           # 16 k-subtiles per job
JOBS = 5                # jobs per core: 0-2 -> W slice A, 3-4 -> W slice B
SLOT_OF_JOB = (0, 0, 0, 1, 1)
SLOT_CAP = (3, 2)

# fallback (expert-per-core) mode
KSUB = K // P           # 32
CAP_FB = 384            # token capacity per launch in fallback mode

_prog_cache: dict[str, object] = {}


def _new_bacc():
    from concourse import bacc

    return bacc.Bacc(
        "TRN2",
        target_bir_lowering=False,
        debug=False,
        num_devices=N_CORES,
    )


def _build_program_jobs():
    import concourse.mybir as mybir
    import concourse.tile as tile

    f16 = mybir.dt.float16
    f32 = mybir.dt.float32

    nc = _new_bacc()
    xj = nc.declare_dram_parameter("xj", [JOBS, KS2, P, P], f16, isOutput=False)
    wh = nc.declare_dram_parameter("wh", [2, KS2, P, H], f16, isOutput=False)
    gs = nc.declare_dram_parameter("gs", [P, JOBS], f32, isOutput=False)
    ho = nc.declare_dram_parameter("ho", [JOBS, P, H], f16, isOutput=True)

    with tile.TileContext(nc) as tc:
        with tc.tile_pool(name="sb", bufs=1) as sb, \
             tc.tile_pool(name="ps", bufs=2, space="PSUM") as psp:
            # W slice A first (jobs 0-2 consume it), then slice B
            wt = [[None] * KS2 for _ in range(2)]
            for s in range(2):
                for ks in range(KS2):
                    w_t = sb.tile([P, H], f16, name=f"w{s}_{ks}",
                                  tag=f"w{s}_{ks}", bufs=1)
                    nc.sync.dma_start(w_t[:], wh[s, ks, :, :])
                    wt[s][ks] = w_t
            xt = [[None] * KS2 for _ in range(JOBS)]
            for j in range(JOBS):
                for ks in range(KS2):
                    x_t = sb.tile([P, P], f16, name=f"x{j}_{ks}",
                                  tag=f"x{j}_{ks}", bufs=1)
                    nc.sync.dma_start(x_t[:], xj[j, ks, :, :])
                    xt[j][ks] = x_t
            # gates: one DMA, then a scalar-engine copy; evictions read the
            # copy so their gate dependency is ACT-engine-local and each
            # eviction activation carries only the PE wait (hardware allows a
            # single sync-wait per instruction; bacc splits the rest).
            g_raw = sb.tile([P, JOBS], f32, name="g_raw", tag="g_raw", bufs=1)
            nc.sync.dma_start(g_raw[:], gs[:, :])
            g2 = sb.tile([P, JOBS], f32, name="g2", tag="g2", bufs=1)
            nc.scalar.copy(g2[:], g_raw[:])

            for j in range(JOBS):
                s = SLOT_OF_JOB[j]
                ps = psp.tile([P, H], f32, name=f"ps{j}", tag="ps", bufs=2)
                for ks in range(KS2):
                    lhs = xt[j][ks][:]              # stationary [K=128, M=128]
                    for h in range(NH):
                        nc.tensor.matmul(
                            ps[:, h * NF:(h + 1) * NF],
                            lhsT=lhs,
                            rhs=wt[s][ks][:, h * NF:(h + 1) * NF],
                            start=(ks == 0),
                            stop=(ks == KS2 - 1),
                        )
                o_t = sb.tile([P, H], f16, name=f"o{j}", tag="o", bufs=JOBS)
                # fp32 gate applied exactly: out = f16(psum_f32 * gate_f32)
                nc.scalar.activation(
                    o_t[:],
                    ps[:],
                    mybir.ActivationFunctionType.Copy,
                    scale=g2[:, j:j + 1],
                )
                nc.sync.dma_start(ho[j, :, :], o_t[:])
    nc.finalize()
    return nc


def _build_program_fallback(cap: int):
    import concourse.mybir as mybir
    import concourse.tile as tile

    f16 = mybir.dt.float16
    f32 = mybir.dt.float32
    ntok = cap // P

    nc = _new_bacc()
    xT = nc.declare_dram_parameter("xT", [KSUB, P, cap], f16, isOutput=False)
    wk = nc.declare_dram_parameter("wk", [KSUB, P, H], f16, isOutput=False)
    gs = nc.declare_dram_parameter("gs", [P, ntok], f32, isOutput=False)
    ho = nc.declare_dram_parameter("ho", [ntok, P, H], f16, isOutput=True)

    with tile.TileContext(nc) as tc:
        with tc.tile_pool(name="sb", bufs=1) as sb, \
             tc.tile_pool(name="ps", bufs=2, space="PSUM") as psp:
            xt, wt = [], []
            for k in range(KSUB):
                x_t = sb.tile([P, cap], f16, name=f"x{k}", tag=f"x{k}", bufs=1)
                nc.sync.dma_start(x_t[:], xT[k, :, :])
                w_t = sb.tile([P, H], f16, name=f"w{k}", tag=f"w{k}", bufs=1)
                nc.sync.dma_start(w_t[:], wk[k, :, :])
                xt.append(x_t)
                wt.append(w_t)
            g_raw = sb.tile([P, ntok], f32, name="g_raw", tag="g_raw", bufs=1)
            nc.sync.dma_start(g_raw[:], gs[:, :])
            g2 = sb.tile([P, ntok], f32, name="g2", tag="g2", bufs=1)
            nc.scalar.copy(g2[:], g_raw[:])

            for t in range(ntok):
                ps = psp.tile([P, H], f32, name=f"ps{t}", tag="ps", bufs=2)
                for k in range(KSUB):
                    lhs = xt[k][:, t * P:(t + 1) * P]
                    for h in range(NH):
                        nc.tensor.matmul(
                            ps[:, h * NF:(h + 1) * NF],
                            lhsT=lhs,
                            rhs=wt[k][:, h * NF:(h + 1) * NF],
                            start=(k == 0),
                            stop=(k == KSUB - 1),
                        )
                o_t = sb.tile([P, H], f16, name=f"o{t}", tag="o", bufs=ntok)
                nc.scalar.activation(
                    o_t[:],
                    ps[:],
                    mybir.ActivationFunctionType.Copy,
                    scale=g2[:, t:t + 1],
                )
                nc.sync.dma_start(ho[t, :, :], o_t[:])
    nc.finalize()
    return nc


def _get_program(key):
    if key not in _prog_cache:
        if key == "jobs":
            _prog_cache[key] = _build_program_jobs()
        else:
            _prog_cache[key] = _build_program_fallback(int(key.split(":")[1]))
    return _prog_cache[key]


def _route(logits, topk):
    """numpy replica of jax.lax.top_k + softmax over selected logits."""
    idx = np.argsort(-logits, axis=-1, kind="stable")[:, :topk]      # [T, topk]
    vals = np.take_along_axis(logits, idx, axis=-1)
    mx = vals.max(-1, keepdims=True)
    gate = np.exp(vals - mx)
    gate = gate / gate.sum(-1, keepdims=True)                        # f32
    return idx, gate


def _pack_groups(tiles_per_expert):
    """Assign (expert, khalf) groups to (core, slot).  Returns
    {(e, kh): (core, slot)} or None if infeasible."""
    groups = []
    for e, ntile in enumerate(tiles_per_expert):
        if ntile == 0:
            continue
        for kh in range(2):
            groups.append((ntile, e, kh))
    groups.sort(reverse=True)
    slots = []  # (capacity, core, slot)
    for c in range(N_CORES):
        slots.append([SLOT_CAP[0], c, 0])
        slots.append([SLOT_CAP[1], c, 1])
    # place largest groups first into the fullest-fitting free slot
    assign = {}
    used = [False] * len(slots)
    for ntile, e, kh in groups:
        best = None
        for i, (cap_s, c, s) in enumerate(slots):
            if used[i] or cap_s < ntile:
                continue
            if best is None or cap_s < slots[best][0]:
                best = i
        if best is None:
            return None
        used[best] = True
        assign[(e, kh)] = (slots[best][1], slots[best][2])
    return assign


def prepare(inputs):
    """Host routing + per-core input construction.

    Returns (nc, launches, combine): launches is a list of per-launch in_maps
    (one dict per core); combine(list_of_per_launch_results) -> final output.
    """
    x = np.asarray(inputs["intermediate_states"])          # [R, TK, I_PR] f16
    w = np.asarray(inputs["w"])                            # [R, E, I_PR, H] f16
    logits = np.asarray(inputs["router_logits"]).astype(np.float32)  # [T, E]
    topk = int(np.asarray(inputs["topk"]))

    T, E_ = logits.shape
    TK = T * topk
    assert x.shape == (R, TK, I_PR) and w.shape == (R, E_, I_PR, H) and E_ == E

    idx, gate = _route(logits, topk)
    flat_e = idx.reshape(-1)                               # expert of tk
    counts = np.bincount(flat_e, minlength=E)
    starts = np.zeros(E + 1, np.int64)
    starts[1:] = np.cumsum(counts)
    order = np.argsort(flat_e, kind="stable")              # tks sorted by expert
    g_flat = gate.reshape(TK)
    xf = np.ascontiguousarray(x.transpose(1, 0, 2)).reshape(TK, K)  # [TK, 4096]

    tiles_per_expert = [-(-int(c) // P) for c in counts]
    assign = _pack_groups(tiles_per_expert)
    if assign is not None:
        return _prepare_jobs(w, xf, g_flat, order, starts, counts,
                             tiles_per_expert, assign, topk, T)
    return _prepare_fallback(w, xf, g_flat, order, starts, counts, topk, T)


def _prepare_jobs(w, xf, g_flat, order, starts, counts, tiles_per_expert,
                  assign, topk, T):
    TK = T * topk
    nc = _get_program("jobs")

    xj = np.zeros((N_CORES, JOBS, KS2, P, P), np.float16)
    whs = np.zeros((N_CORES, 2, KS2, P, H), np.float16)
    gss = np.zeros((N_CORES, P, JOBS), np.float32)
    # pos[kh][tk] = row index in the assembled h for token tk's kh-half partial
    pos = np.zeros((2, TK), np.int64)

    job_base = {0: 0, 1: SLOT_CAP[0]}
    for (e, kh), (core, slot) in assign.items():
        toks_e = order[starts[e]:starts[e + 1]]            # ascending tks
        c_e = int(counts[e])
        # w slice: K-half kh of expert e -> [2048, 2048]
        wsl = np.ascontiguousarray(w[2 * kh:2 * kh + 2, e].reshape(KH, H))
        whs[core, slot] = wsl.reshape(KS2, P, H)
        for tt in range(tiles_per_expert[e]):
            j = job_base[slot] + tt
            toks = toks_e[tt * P:(tt + 1) * P]
            n = len(toks)
            # stationary tiles: [KS2, P(k), P(tok)] from x K-half kh
            xs = xf[toks, kh * KH:(kh + 1) * KH]           # [n, 2048] f16
            xj[core, j, :, :, :n] = xs.T.reshape(KS2, P, n)
            gss[core, :n, j] = g_flat[toks]
            pos[kh, toks] = (core * JOBS + j) * P + np.arange(n)

    launches = [[{"xj": xj[c], "wh": whs[c], "gs": gss[c]}
                 for c in range(N_CORES)]]

    def combine(all_results):
        res = all_results[0]
        h_all = np.concatenate(
            [res[c]["ho"].reshape(JOBS * P, H) for c in range(N_CORES)], axis=0)
        y = np.zeros((T, H), np.float32)
        for kh in range(2):
            for kk in range(topk):
                y += h_all[pos[kh, kk::topk]].astype(np.float32)
        return y.astype(np.float16).reshape(R, T // R, H)

    return nc, launches, combine


def _prepare_fallback(w, xf, g_flat, order, starts, counts, topk, T):
    TK = T * topk
    cap_needed = -(-max(int(counts.max()), 1) // P) * P
    cap_launch = min(cap_needed, CAP_FB)
    n_launch = -(-cap_needed // cap_launch)
    cap_total = n_launch * cap_launch
    ntok_l = cap_launch // P

    nc = _get_program(f"fb:{cap_launch}")

    flat_starts = starts[:-1]
    pos = np.empty(TK, np.int64)
    for e in range(E):
        toks = order[starts[e]:starts[e + 1]]
        pos[toks] = e * cap_total + np.arange(len(toks))

    launches = []
    for j in range(n_launch):
        in_maps = []
        for e in range(E):
            toks = order[starts[e]:starts[e + 1]][j * cap_launch:(j + 1) * cap_launch]
            c = len(toks)
            xTe = np.zeros((K, cap_launch), np.float16)
            gse = np.zeros((cap_launch,), np.float32)
            if c:
                xTe[:, :c] = xf[toks].T
                gse[:c] = g_flat[toks]
            in_maps.append({
                "xT": np.ascontiguousarray(xTe.reshape(KSUB, P, cap_launch)),
                "wk": np.ascontiguousarray(w[:, e].reshape(K, H)).reshape(KSUB, P, H),
                "gs": np.ascontiguousarray(gse.reshape(ntok_l, P).T),
            })
        launches.append(in_maps)

    def combine(all_results):
        h_all = np.empty((E * cap_total, H), np.float16)
        for j, res in enumerate(all_results):
            for e in range(E):
                h_all[e * cap_total + j * cap_launch:
                      e * cap_total + (j + 1) * cap_launch] = \
                    res[e]["ho"].reshape(cap_launch, H)
        y = h_all[pos[0::topk]].astype(np.float32)
        for kk in range(1, topk):
            y += h_all[pos[kk::topk]].astype(np.float32)
        return y.astype(np.float16).reshape(R, T // R, H)

    return nc, launches, combine


def kernel(**inputs) -> np.ndarray:
    nc, launches, combine = prepare(inputs)
    from concourse.bass_utils import run_bass_kernel_spmd

    all_results = []
    for in_maps in launches:
        res = run_bass_kernel_spmd(nc, in_maps, core_ids=list(range(N_CORES)))
        all_results.append(res.results)
    return combine(all_results)


# revision 13
# speedup vs baseline: 1.1396x; 1.1396x over previous
"""MoE top-k routing + grouped down-proj GEMM + reduce-scatter for trn2 (8 cores).

Problem: intermediate_states [4, 2048, 1024] f16 (rank-sharded expanded-token
activations), w [4, 8, 1024, 2048] f16 (rank-sharded per-expert down-proj),
router_logits [1024, 8] f32, topk=2.  Output [4, 256, 2048] f16.

Strategy: the sparse compute is, per expanded token tk routed to expert e(tk):
y_part[tk] = gate(tk) * (x_full[tk] @ w_full[e(tk)]) with x_full [TK, 4096]
(rank dim folded into the contraction) and w_full[e] [4096, 2048].

Work is decomposed into jobs of (one 128-token tile of one expert) x (one
K-half of 2048).  Each core runs JOBS=5 such jobs: jobs 0-2 share W-slice A,
jobs 3-4 share W-slice B, where a slice is one (expert, K-half) [2048, 2048]
f16 weight block.  The host packs the (expert, K-half) groups (each of
ceil(c_e/128) jobs) into the 8 cores' A/B slots; with sum(ceil(c_e/128)) <= 23
and all c_e <= 384 this always fits, giving near-perfectly balanced PE load
(5 * 16 * 4 matmuls of [128,128]@[128,512] per core).  Each job accumulates in
fp32 PSUM over 16 K-subtiles and applies its fp32 routing gate as a
per-partition scale at PSUM eviction.  No collective is needed: the host sums
each final token's 4 partial rows (topk=2 experts x 2 K-halves).  Routing /
top-k / softmax runs on host in numpy (1024x8 logits - negligible).

Fallback: if some expert has more than 384 tokens the job packing doesn't fit;
an expert-per-core kernel (full K=4096, capacity padded to 128) is used
instead, launched as many times as needed.
"""

import numpy as np

R, T_TOK, TOPK, E = 4, 1024, 2, 8
I_PR, H = 1024, 2048
K = R * I_PR            # 4096 contraction
P = 128
NF = 512                # matmul free-dim (one PSUM bank of fp32)
NH = H // NF            # 4
N_CORES = 8

# job mode
KH = K // 2             # 2048 per K-half
KS2 = KH // P           # 16 k-subtiles per job
JOBS = 5                # jobs per core: 0-2 -> W slice A, 3-4 -> W slice B
SLOT_OF_JOB = (0, 0, 0, 1, 1)
SLOT_CAP = (3, 2)

# fallback (expert-per-core) mode
KSUB = K // P           # 32
CAP_FB = 384            # token capacity per launch in fallback mode

_prog_cache: dict[str, object] = {}


def _new_bacc():
    from concourse import bacc

    return bacc.Bacc(
        "TRN2",
        target_bir_lowering=False,
        debug=False,
        num_devices=N_CORES,
    )


def _build_program_jobs():
    import concourse.mybir as mybir
    import concourse.tile as tile

    f16 = mybir.dt.float16
    f32 = mybir.dt.float32

    nc = _new_bacc()
    xj = nc.declare_dram_parameter("xj", [JOBS, KS2, P, P], f16, isOutput=False)
    wh = nc.declare_dram_parameter("wh", [2, KS2, P, H], f16, isOutput=False)
    gs = nc.declare_dram_parameter("gs", [P, JOBS], f32, isOutput=False)
    ho = nc.declare_dram_parameter("ho", [JOBS, P, H], f16, isOutput=True)

    with tile.TileContext(nc) as tc:
        with tc.tile_pool(name="sb", bufs=1) as sb, \
             tc.tile_pool(name="ps", bufs=2, space="PSUM") as psp:
            # DMA issue order feeds the PE's actual consumption order:
            # stationary x tiles + gates first (2.6MB), then W slice A
            # (jobs 0-2), then W slice B (jobs 3-4).
            xt = [[None] * KS2 for _ in range(JOBS)]
            for j in range(JOBS):
                for ks in range(KS2):
                    x_t = sb.tile([P, P], f16, name=f"x{j}_{ks}",
                                  tag=f"x{j}_{ks}", bufs=1)
                    nc.sync.dma_start(x_t[:], xj[j, ks, :, :])
                    xt[j][ks] = x_t
            # gates: one DMA, then a scalar-engine copy; evictions read the
            # copy so their gate dependency is ACT-engine-local and each
            # eviction activation carries only the PE wait (hardware allows a
            # single sync-wait per instruction; bacc splits the rest).
            g_raw = sb.tile([P, JOBS], f32, name="g_raw", tag="g_raw", bufs=1)
            nc.sync.dma_start(g_raw[:], gs[:, :])
            g2 = sb.tile([P, JOBS], f32, name="g2", tag="g2", bufs=1)
            nc.scalar.copy(g2[:], g_raw[:])

            wt = [[None] * KS2 for _ in range(2)]
            for s in range(2):
                for ks in range(KS2):
                    w_t = sb.tile([P, H], f16, name=f"w{s}_{ks}",
                                  tag=f"w{s}_{ks}", bufs=1)
                    nc.sync.dma_start(w_t[:], wh[s, ks, :, :])
                    wt[s][ks] = w_t

            ps_tiles = {}

            def open_job(j):
                ps_tiles[j] = psp.tile([P, H], f32, name=f"ps{j}", tag="ps",
                                       bufs=2)

            def mm(j, ks):
                s = SLOT_OF_JOB[j]
                lhs = xt[j][ks][:]                  # stationary [K=128, M=128]
                for h in range(NH):
                    nc.tensor.matmul(
                        ps_tiles[j][:, h * NF:(h + 1) * NF],
                        lhsT=lhs,
                        rhs=wt[s][ks][:, h * NF:(h + 1) * NF],
                        start=(ks == 0),
                        stop=(ks == KS2 - 1),
                    )

            def evict(j):
                o_t = sb.tile([P, H], f16, name=f"o{j}", tag="o", bufs=JOBS)
                # fp32 gate applied exactly: out = f16(psum_f32 * gate_f32)
                nc.scalar.activation(
                    o_t[:],
                    ps_tiles[j][:],
                    mybir.ActivationFunctionType.Copy,
                    scale=g2[:, j:j + 1],
                )
                nc.sync.dma_start(ho[j, :, :], o_t[:])

            # jobs 0+1 interleaved per chunk (8 matmuls per arriving W chunk
            # keeps the PE ahead of the DMA stream), then job 2 on the
            # now-resident slice A, then jobs 3+4 interleaved on slice B.
            for j in (0, 1):
                open_job(j)
            for ks in range(KS2):
                for j in (0, 1):
                    mm(j, ks)
            for j in (0, 1):
                evict(j)
            open_job(2)
            for ks in range(KS2):
                mm(2, ks)
            evict(2)
            for j in (3, 4):
                open_job(j)
            for ks in range(KS2):
                for j in (3, 4):
                    mm(j, ks)
            for j in (3, 4):
                evict(j)
    nc.finalize()
    return nc


def _build_program_fallback(cap: int):
    import concourse.mybir as mybir
    import concourse.tile as tile

    f16 = mybir.dt.float16
    f32 = mybir.dt.float32
    ntok = cap // P

    nc = _new_bacc()
    xT = nc.declare_dram_parameter("xT", [KSUB, P, cap], f16, isOutput=False)
    wk = nc.declare_dram_parameter("wk", [KSUB, P, H], f16, isOutput=False)
    gs = nc.declare_dram_parameter("gs", [P, ntok], f32, isOutput=False)
    ho = nc.declare_dram_parameter("ho", [ntok, P, H], f16, isOutput=True)

    with tile.TileContext(nc) as tc:
        with tc.tile_pool(name="sb", bufs=1) as sb, \
             tc.tile_pool(name="ps", bufs=2, space="PSUM") as psp:
            xt, wt = [], []
            for k in range(KSUB):
                x_t = sb.tile([P, cap], f16, name=f"x{k}", tag=f"x{k}", bufs=1)
                nc.sync.dma_start(x_t[:], xT[k, :, :])
                w_t = sb.tile([P, H], f16, name=f"w{k}", tag=f"w{k}", bufs=1)
                nc.sync.dma_start(w_t[:], wk[k, :, :])
                xt.append(x_t)
                wt.append(w_t)
            g_raw = sb.tile([P, ntok], f32, name="g_raw", tag="g_raw", bufs=1)
            nc.sync.dma_start(g_raw[:], gs[:, :])
            g2 = sb.tile([P, ntok], f32, name="g2", tag="g2", bufs=1)
            nc.scalar.copy(g2[:], g_raw[:])

            for t in range(ntok):
                ps = psp.tile([P, H], f32, name=f"ps{t}", tag="ps", bufs=2)
                for k in range(KSUB):
                    lhs = xt[k][:, t * P:(t + 1) * P]
                    for h in range(NH):
                        nc.tensor.matmul(
                            ps[:, h * NF:(h + 1) * NF],
                            lhsT=lhs,
                            rhs=wt[k][:, h * NF:(h + 1) * NF],
                            start=(k == 0),
                            stop=(k == KSUB - 1),
                        )
                o_t = sb.tile([P, H], f16, name=f"o{t}", tag="o", bufs=ntok)
                nc.scalar.activation(
                    o_t[:],
                    ps[:],
                    mybir.ActivationFunctionType.Copy,
                    scale=g2[:, t:t + 1],
                )
                nc.sync.dma_start(ho[t, :, :], o_t[:])
    nc.finalize()
    return nc


def _get_program(key):
    if key not in _prog_cache:
        if key == "jobs":
            _prog_cache[key] = _build_program_jobs()
        else:
            _prog_cache[key] = _build_program_fallback(int(key.split(":")[1]))
    return _prog_cache[key]


def _route(logits, topk):
    """numpy replica of jax.lax.top_k + softmax over selected logits."""
    idx = np.argsort(-logits, axis=-1, kind="stable")[:, :topk]      # [T, topk]
    vals = np.take_along_axis(logits, idx, axis=-1)
    mx = vals.max(-1, keepdims=True)
    gate = np.exp(vals - mx)
    gate = gate / gate.sum(-1, keepdims=True)                        # f32
    return idx, gate


def _pack_groups(tiles_per_expert):
    """Assign (expert, khalf) groups to (core, slot).  Returns
    {(e, kh): (core, slot)} or None if infeasible."""
    groups = []
    for e, ntile in enumerate(tiles_per_expert):
        if ntile == 0:
            continue
        for kh in range(2):
            groups.append((ntile, e, kh))
    groups.sort(reverse=True)
    slots = []  # (capacity, core, slot)
    for c in range(N_CORES):
        slots.append([SLOT_CAP[0], c, 0])
        slots.append([SLOT_CAP[1], c, 1])
    # place largest groups first into the fullest-fitting free slot
    assign = {}
    used = [False] * len(slots)
    for ntile, e, kh in groups:
        best = None
        for i, (cap_s, c, s) in enumerate(slots):
            if used[i] or cap_s < ntile:
                continue
            if best is None or cap_s < slots[best][0]:
                best = i
        if best is None:
            return None
        used[best] = True
        assign[(e, kh)] = (slots[best][1], slots[best][2])
    return assign


def prepare(inputs):
    """Host routing + per-core input construction.

    Returns (nc, launches, combine): launches is a list of per-launch in_maps
    (one dict per core); combine(list_of_per_launch_results) -> final output.
    """
    x = np.asarray(inputs["intermediate_states"])          # [R, TK, I_PR] f16
    w = np.asarray(inputs["w"])                            # [R, E, I_PR, H] f16
    logits = np.asarray(inputs["router_logits"]).astype(np.float32)  # [T, E]
    topk = int(np.asarray(inputs["topk"]))

    T, E_ = logits.shape
    TK = T * topk
    assert x.shape == (R, TK, I_PR) and w.shape == (R, E_, I_PR, H) and E_ == E

    idx, gate = _route(logits, topk)
    flat_e = idx.reshape(-1)                               # expert of tk
    counts = np.bincount(flat_e, minlength=E)
    starts = np.zeros(E + 1, np.int64)
    starts[1:] = np.cumsum(counts)
    order = np.argsort(flat_e, kind="stable")              # tks sorted by expert
    g_flat = gate.reshape(TK)
    xf = np.ascontiguousarray(x.transpose(1, 0, 2)).reshape(TK, K)  # [TK, 4096]

    tiles_per_expert = [-(-int(c) // P) for c in counts]
    assign = _pack_groups(tiles_per_expert)
    if assign is not None:
        return _prepare_jobs(w, xf, g_flat, order, starts, counts,
                             tiles_per_expert, assign, topk, T)
    return _prepare_fallback(w, xf, g_flat, order, starts, counts, topk, T)


def _prepare_jobs(w, xf, g_flat, order, starts, counts, tiles_per_expert,
                  assign, topk, T):
    TK = T * topk
    nc = _get_program("jobs")

    xj = np.zeros((N_CORES, JOBS, KS2, P, P), np.float16)
    whs = np.zeros((N_CORES, 2, KS2, P, H), np.float16)
    gss = np.zeros((N_CORES, P, JOBS), np.float32)
    # pos[kh][tk] = row index in the assembled h for token tk's kh-half partial
    pos = np.zeros((2, TK), np.int64)

    job_base = {0: 0, 1: SLOT_CAP[0]}
    for (e, kh), (core, slot) in assign.items():
        toks_e = order[starts[e]:starts[e + 1]]            # ascending tks
        c_e = int(counts[e])
        # w slice: K-half kh of expert e -> [2048, 2048]
        wsl = np.ascontiguousarray(w[2 * kh:2 * kh + 2, e].reshape(KH, H))
        whs[core, slot] = wsl.reshape(KS2, P, H)
        for tt in range(tiles_per_expert[e]):
            j = job_base[slot] + tt
            toks = toks_e[tt * P:(tt + 1) * P]
            n = len(toks)
            # stationary tiles: [KS2, P(k), P(tok)] from x K-half kh
            xs = xf[toks, kh * KH:(kh + 1) * KH]           # [n, 2048] f16
            xj[core, j, :, :, :n] = xs.T.reshape(KS2, P, n)
            gss[core, :n, j] = g_flat[toks]
            pos[kh, toks] = (core * JOBS + j) * P + np.arange(n)

    launches = [[{"xj": xj[c], "wh": whs[c], "gs": gss[c]}
                 for c in range(N_CORES)]]

    def combine(all_results):
        res = all_results[0]
        h_all = np.concatenate(
            [res[c]["ho"].reshape(JOBS * P, H) for c in range(N_CORES)], axis=0)
        y = np.zeros((T, H), np.float32)
        for kh in range(2):
            for kk in range(topk):
                y += h_all[pos[kh, kk::topk]].astype(np.float32)
        return y.astype(np.float16).reshape(R, T // R, H)

    return nc, launches, combine


def _prepare_fallback(w, xf, g_flat, order, starts, counts, topk, T):
    TK = T * topk
    cap_needed = -(-max(int(counts.max()), 1) // P) * P
    cap_launch = min(cap_needed, CAP_FB)
    n_launch = -(-cap_needed // cap_launch)
    cap_total = n_launch * cap_launch
    ntok_l = cap_launch // P

    nc = _get_program(f"fb:{cap_launch}")

    flat_starts = starts[:-1]
    pos = np.empty(TK, np.int64)
    for e in range(E):
        toks = order[starts[e]:starts[e + 1]]
        pos[toks] = e * cap_total + np.arange(len(toks))

    launches = []
    for j in range(n_launch):
        in_maps = []
        for e in range(E):
            toks = order[starts[e]:starts[e + 1]][j * cap_launch:(j + 1) * cap_launch]
            c = len(toks)
            xTe = np.zeros((K, cap_launch), np.float16)
            gse = np.zeros((cap_launch,), np.float32)
            if c:
                xTe[:, :c] = xf[toks].T
                gse[:c] = g_flat[toks]
            in_maps.append({
                "xT": np.ascontiguousarray(xTe.reshape(KSUB, P, cap_launch)),
                "wk": np.ascontiguousarray(w[:, e].reshape(K, H)).reshape(KSUB, P, H),
                "gs": np.ascontiguousarray(gse.reshape(ntok_l, P).T),
            })
        launches.append(in_maps)

    def combine(all_results):
        h_all = np.empty((E * cap_total, H), np.float16)
        for j, res in enumerate(all_results):
            for e in range(E):
                h_all[e * cap_total + j * cap_launch:
                      e * cap_total + (j + 1) * cap_launch] = \
                    res[e]["ho"].reshape(cap_launch, H)
        y = h_all[pos[0::topk]].astype(np.float32)
        for kk in range(1, topk):
            y += h_all[pos[kk::topk]].astype(np.float32)
        return y.astype(np.float16).reshape(R, T // R, H)

    return nc, launches, combine


def kernel(**inputs) -> np.ndarray:
    nc, launches, combine = prepare(inputs)
    from concourse.bass_utils import run_bass_kernel_spmd

    all_results = []
    for in_maps in launches:
        res = run_bass_kernel_spmd(nc, in_maps, core_ids=list(range(N_CORES)))
        all_results.append(res.results)
    return combine(all_results)


# revision 20
# speedup vs baseline: 1.6102x; 1.4129x over previous
"""MoE top-k routing + grouped down-proj GEMM + reduce-scatter for trn2 (8 cores).

Problem: intermediate_states [4, 2048, 1024] f16 (rank-sharded expanded-token
activations), w [4, 8, 1024, 2048] f16 (rank-sharded per-expert down-proj),
router_logits [1024, 8] f32, topk=2.  Output [4, 256, 2048] f16.

Strategy: the sparse compute is, per expanded token tk routed to expert e(tk):
y_part[tk] = gate(tk) * (x_full[tk] @ w_full[e(tk)]) with x_full [TK, 4096]
(rank dim folded into the contraction) and w_full[e] [4096, 2048].

Work is decomposed into jobs of (one 128-token tile of one expert) x (one
K-half of 2048).  Each core runs JOBS=5 such jobs: jobs 0-2 share W-slice A,
jobs 3-4 share W-slice B, where a slice is one (expert, K-half) [2048, 2048]
f16 weight block.  The host packs the (expert, K-half) groups (each of
ceil(c_e/128) jobs) into the 8 cores' A/B slots; with sum(ceil(c_e/128)) <= 23
and all c_e <= 384 this always fits, giving near-perfectly balanced PE load
(5 * 16 * 4 matmuls of [128,128]@[128,512] per core).  Each job accumulates in
fp32 PSUM over 16 K-subtiles and applies its fp32 routing gate as a
per-partition scale at PSUM eviction.  No collective is needed: the host sums
each final token's 4 partial rows (topk=2 experts x 2 K-halves).  Routing /
top-k / softmax runs on host in numpy (1024x8 logits - negligible).

Fallback: if some expert has more than 384 tokens the job packing doesn't fit;
an expert-per-core kernel (full K=4096, capacity padded to 128) is used
instead, launched as many times as needed.
"""

import numpy as np

R, T_TOK, TOPK, E = 4, 1024, 2, 8
I_PR, H = 1024, 2048
K = R * I_PR            # 4096 contraction
P = 128
NF = 512                # matmul free-dim (one PSUM bank of fp32)
NH = H // NF            # 4
N_CORES = 8

# job mode
KH = K // 2             # 2048 per K-half
KS2 = KH // P           # 16 k-subtiles per job
JOBS = 5                # jobs per core: 0-2 -> W slice A, 3-4 -> W slice B
SLOT_OF_JOB = (0, 0, 0, 1, 1)
SLOT_CAP = (3, 2)

# fallback (expert-per-core) mode
KSUB = K // P           # 32
CAP_FB = 384            # token capacity per launch in fallback mode

_prog_cache: dict[str, object] = {}


def _new_bacc():
    from concourse import bacc

    return bacc.Bacc(
        "TRN2",
        target_bir_lowering=False,
        debug=False,
        num_devices=N_CORES,
    )


def _build_program_jobs():
    import concourse.mybir as mybir
    import concourse.tile as tile

    f16 = mybir.dt.float16
    f32 = mybir.dt.float32

    nc = _new_bacc()
    # xj[j, p, ks*P + m] = x value of job-j token m at K-row ks*P + p of the
    # job's K-half: exactly the SBUF stationary layout, so loading is a plain
    # 2D DMA per job.
    xj = nc.declare_dram_parameter("xj", [JOBS, P, KS2 * P], f16, isOutput=False)
    wh = nc.declare_dram_parameter("wh", [2, KS2, P, H], f16, isOutput=False)
    gs = nc.declare_dram_parameter("gs", [P, JOBS], f32, isOutput=False)
    ho = nc.declare_dram_parameter("ho", [JOBS, P, H], f16, isOutput=True)

    with tile.TileContext(nc) as tc:
        with tc.tile_pool(name="sb", bufs=1) as sb, \
             tc.tile_pool(name="ps", bufs=2, space="PSUM") as psp:
            # DMA issue order feeds the PE's actual consumption order:
            # stationary x tiles + gates first (2.6MB), then W slice A
            # (jobs 0-2), then W slice B (jobs 3-4).
            # one plain 2D DMA per job (per-DMA issue on the sync queue is
            # ~0.65us regardless of size, so many small stationary-tile DMAs
            # would serialize for tens of microseconds before the W stream
            # even starts issuing)
            xt = []
            for j in range(JOBS):
                x_t = sb.tile([P, KS2 * P], f16, name=f"x{j}", tag=f"x{j}",
                              bufs=1)
                nc.sync.dma_start(x_t[:], xj[j, :, :])
                xt.append(x_t)
            # gates: one DMA, then a scalar-engine copy; evictions read the
            # copy so their gate dependency is ACT-engine-local and each
            # eviction activation carries only the PE wait (hardware allows a
            # single sync-wait per instruction; bacc splits the rest).
            g_raw = sb.tile([P, JOBS], f32, name="g_raw", tag="g_raw", bufs=1)
            nc.sync.dma_start(g_raw[:], gs[:, :])
            g2 = sb.tile([P, JOBS], f32, name="g2", tag="g2", bufs=1)
            nc.scalar.copy(g2[:], g_raw[:])

            wt = [[None] * KS2 for _ in range(2)]
            for s in range(2):
                for ks in range(KS2):
                    w_t = sb.tile([P, H], f16, name=f"w{s}_{ks}",
                                  tag=f"w{s}_{ks}", bufs=1)
                    nc.sync.dma_start(w_t[:], wh[s, ks, :, :])
                    wt[s][ks] = w_t

            ps_tiles = {}

            def open_job(j):
                ps_tiles[j] = psp.tile([P, H], f32, name=f"ps{j}", tag="ps",
                                       bufs=2)

            def mm(j, ks):
                s = SLOT_OF_JOB[j]
                lhs = xt[j][:, ks * P:(ks + 1) * P]  # stationary [K=128, M=128]
                for h in range(NH):
                    nc.tensor.matmul(
                        ps_tiles[j][:, h * NF:(h + 1) * NF],
                        lhsT=lhs,
                        rhs=wt[s][ks][:, h * NF:(h + 1) * NF],
                        start=(ks == 0),
                        stop=(ks == KS2 - 1),
                    )

            def evict(j):
                o_t = sb.tile([P, H], f16, name=f"o{j}", tag="o", bufs=JOBS)
                # fp32 gate applied exactly: out = f16(psum_f32 * gate_f32)
                nc.scalar.activation(
                    o_t[:],
                    ps_tiles[j][:],
                    mybir.ActivationFunctionType.Copy,
                    scale=g2[:, j:j + 1],
                )
                nc.sync.dma_start(ho[j, :, :], o_t[:])

            # jobs 0+1 interleaved per chunk (8 matmuls per arriving W chunk
            # keeps the PE ahead of the DMA stream), then job 2 on the
            # now-resident slice A, then jobs 3+4 interleaved on slice B.
            for j in (0, 1):
                open_job(j)
            for ks in range(KS2):
                for j in (0, 1):
                    mm(j, ks)
            for j in (0, 1):
                evict(j)
            open_job(2)
            for ks in range(KS2):
                mm(2, ks)
            evict(2)
            for j in (3, 4):
                open_job(j)
            for ks in range(KS2):
                for j in (3, 4):
                    mm(j, ks)
            for j in (3, 4):
                evict(j)
    nc.finalize()
    return nc


def _build_program_fallback(cap: int):
    import concourse.mybir as mybir
    import concourse.tile as tile

    f16 = mybir.dt.float16
    f32 = mybir.dt.float32
    ntok = cap // P

    nc = _new_bacc()
    xT = nc.declare_dram_parameter("xT", [KSUB, P, cap], f16, isOutput=False)
    wk = nc.declare_dram_parameter("wk", [KSUB, P, H], f16, isOutput=False)
    gs = nc.declare_dram_parameter("gs", [P, ntok], f32, isOutput=False)
    ho = nc.declare_dram_parameter("ho", [ntok, P, H], f16, isOutput=True)

    with tile.TileContext(nc) as tc:
        with tc.tile_pool(name="sb", bufs=1) as sb, \
             tc.tile_pool(name="ps", bufs=2, space="PSUM") as psp:
            xt, wt = [], []
            for k in range(KSUB):
                x_t = sb.tile([P, cap], f16, name=f"x{k}", tag=f"x{k}", bufs=1)
                nc.sync.dma_start(x_t[:], xT[k, :, :])
                w_t = sb.tile([P, H], f16, name=f"w{k}", tag=f"w{k}", bufs=1)
                nc.sync.dma_start(w_t[:], wk[k, :, :])
                xt.append(x_t)
                wt.append(w_t)
            g_raw = sb.tile([P, ntok], f32, name="g_raw", tag="g_raw", bufs=1)
            nc.sync.dma_start(g_raw[:], gs[:, :])
            g2 = sb.tile([P, ntok], f32, name="g2", tag="g2", bufs=1)
            nc.scalar.copy(g2[:], g_raw[:])

            for t in range(ntok):
                ps = psp.tile([P, H], f32, name=f"ps{t}", tag="ps", bufs=2)
                for k in range(KSUB):
                    lhs = xt[k][:, t * P:(t + 1) * P]
                    for h in range(NH):
                        nc.tensor.matmul(
                            ps[:, h * NF:(h + 1) * NF],
                            lhsT=lhs,
                            rhs=wt[k][:, h * NF:(h + 1) * NF],
                            start=(k == 0),
                            stop=(k == KSUB - 1),
                        )
                o_t = sb.tile([P, H], f16, name=f"o{t}", tag="o", bufs=ntok)
                nc.scalar.activation(
                    o_t[:],
                    ps[:],
                    mybir.ActivationFunctionType.Copy,
                    scale=g2[:, t:t + 1],
                )
                nc.sync.dma_start(ho[t, :, :], o_t[:])
    nc.finalize()
    return nc


def _get_program(key):
    if key not in _prog_cache:
        if key == "jobs":
            _prog_cache[key] = _build_program_jobs()
        else:
            _prog_cache[key] = _build_program_fallback(int(key.split(":")[1]))
    return _prog_cache[key]


def _route(logits, topk):
    """numpy replica of jax.lax.top_k + softmax over selected logits."""
    idx = np.argsort(-logits, axis=-1, kind="stable")[:, :topk]      # [T, topk]
    vals = np.take_along_axis(logits, idx, axis=-1)
    mx = vals.max(-1, keepdims=True)
    gate = np.exp(vals - mx)
    gate = gate / gate.sum(-1, keepdims=True)                        # f32
    return idx, gate


def _pack_groups(tiles_per_expert):
    """Assign (expert, khalf) groups to (core, slot).  Returns
    {(e, kh): (core, slot)} or None if infeasible."""
    groups = []
    for e, ntile in enumerate(tiles_per_expert):
        if ntile == 0:
            continue
        for kh in range(2):
            groups.append((ntile, e, kh))
    groups.sort(reverse=True)
    slots = []  # (capacity, core, slot)
    for c in range(N_CORES):
        slots.append([SLOT_CAP[0], c, 0])
        slots.append([SLOT_CAP[1], c, 1])
    # place largest groups first into the fullest-fitting free slot
    assign = {}
    used = [False] * len(slots)
    for ntile, e, kh in groups:
        best = None
        for i, (cap_s, c, s) in enumerate(slots):
            if used[i] or cap_s < ntile:
                continue
            if best is None or cap_s < slots[best][0]:
                best = i
        if best is None:
            return None
        used[best] = True
        assign[(e, kh)] = (slots[best][1], slots[best][2])
    return assign


def prepare(inputs):
    """Host routing + per-core input construction.

    Returns (nc, launches, combine): launches is a list of per-launch in_maps
    (one dict per core); combine(list_of_per_launch_results) -> final output.
    """
    x = np.asarray(inputs["intermediate_states"])          # [R, TK, I_PR] f16
    w = np.asarray(inputs["w"])                            # [R, E, I_PR, H] f16
    logits = np.asarray(inputs["router_logits"]).astype(np.float32)  # [T, E]
    topk = int(np.asarray(inputs["topk"]))

    T, E_ = logits.shape
    TK = T * topk
    assert x.shape == (R, TK, I_PR) and w.shape == (R, E_, I_PR, H) and E_ == E

    idx, gate = _route(logits, topk)
    flat_e = idx.reshape(-1)                               # expert of tk
    counts = np.bincount(flat_e, minlength=E)
    starts = np.zeros(E + 1, np.int64)
    starts[1:] = np.cumsum(counts)
    order = np.argsort(flat_e, kind="stable")              # tks sorted by expert
    g_flat = gate.reshape(TK)
    xf = np.ascontiguousarray(x.transpose(1, 0, 2)).reshape(TK, K)  # [TK, 4096]

    tiles_per_expert = [-(-int(c) // P) for c in counts]
    assign = _pack_groups(tiles_per_expert)
    if assign is not None:
        return _prepare_jobs(w, xf, g_flat, order, starts, counts,
                             tiles_per_expert, assign, topk, T)
    return _prepare_fallback(w, xf, g_flat, order, starts, counts, topk, T)


def _prepare_jobs(w, xf, g_flat, order, starts, counts, tiles_per_expert,
                  assign, topk, T):
    TK = T * topk
    nc = _get_program("jobs")

    xj = np.zeros((N_CORES, JOBS, P, KS2, P), np.float16)
    whs = np.zeros((N_CORES, 2, KS2, P, H), np.float16)
    gss = np.zeros((N_CORES, P, JOBS), np.float32)
    # pos[kh][tk] = row index in the assembled h for token tk's kh-half partial
    pos = np.zeros((2, TK), np.int64)

    job_base = {0: 0, 1: SLOT_CAP[0]}
    for (e, kh), (core, slot) in assign.items():
        toks_e = order[starts[e]:starts[e + 1]]            # ascending tks
        c_e = int(counts[e])
        # w slice: K-half kh of expert e -> [2048, 2048]
        wsl = np.ascontiguousarray(w[2 * kh:2 * kh + 2, e].reshape(KH, H))
        whs[core, slot] = wsl.reshape(KS2, P, H)
        for tt in range(tiles_per_expert[e]):
            j = job_base[slot] + tt
            toks = toks_e[tt * P:(tt + 1) * P]
            n = len(toks)
            # stationary layout [P(krow), KS2, P(tok)] from x K-half kh
            xs = xf[toks, kh * KH:(kh + 1) * KH]           # [n, 2048] f16
            xj[core, j, :, :, :n] = xs.reshape(n, KS2, P).transpose(2, 1, 0)
            gss[core, :n, j] = g_flat[toks]
            pos[kh, toks] = (core * JOBS + j) * P + np.arange(n)

    launches = [[{"xj": xj[c].reshape(JOBS, P, KS2 * P), "wh": whs[c],
                  "gs": gss[c]} for c in range(N_CORES)]]

    def combine(all_results):
        res = all_results[0]
        h_all = np.concatenate(
            [res[c]["ho"].reshape(JOBS * P, H) for c in range(N_CORES)], axis=0)
        y = np.zeros((T, H), np.float32)
        for kh in range(2):
            for kk in range(topk):
                y += h_all[pos[kh, kk::topk]].astype(np.float32)
        return y.astype(np.float16).reshape(R, T // R, H)

    return nc, launches, combine


def _prepare_fallback(w, xf, g_flat, order, starts, counts, topk, T):
    TK = T * topk
    cap_needed = -(-max(int(counts.max()), 1) // P) * P
    cap_launch = min(cap_needed, CAP_FB)
    n_launch = -(-cap_needed // cap_launch)
    cap_total = n_launch * cap_launch
    ntok_l = cap_launch // P

    nc = _get_program(f"fb:{cap_launch}")

    flat_starts = starts[:-1]
    pos = np.empty(TK, np.int64)
    for e in range(E):
        toks = order[starts[e]:starts[e + 1]]
        pos[toks] = e * cap_total + np.arange(len(toks))

    launches = []
    for j in range(n_launch):
        in_maps = []
        for e in range(E):
            toks = order[starts[e]:starts[e + 1]][j * cap_launch:(j + 1) * cap_launch]
            c = len(toks)
            xTe = np.zeros((K, cap_launch), np.float16)
            gse = np.zeros((cap_launch,), np.float32)
            if c:
                xTe[:, :c] = xf[toks].T
                gse[:c] = g_flat[toks]
            in_maps.append({
                "xT": np.ascontiguousarray(xTe.reshape(KSUB, P, cap_launch)),
                "wk": np.ascontiguousarray(w[:, e].reshape(K, H)).reshape(KSUB, P, H),
                "gs": np.ascontiguousarray(gse.reshape(ntok_l, P).T),
            })
        launches.append(in_maps)

    def combine(all_results):
        h_all = np.empty((E * cap_total, H), np.float16)
        for j, res in enumerate(all_results):
            for e in range(E):
                h_all[e * cap_total + j * cap_launch:
                      e * cap_total + (j + 1) * cap_launch] = \
                    res[e]["ho"].reshape(cap_launch, H)
        y = h_all[pos[0::topk]].astype(np.float32)
        for kk in range(1, topk):
            y += h_all[pos[kk::topk]].astype(np.float32)
        return y.astype(np.float16).reshape(R, T // R, H)

    return nc, launches, combine


def kernel(**inputs) -> np.ndarray:
    nc, launches, combine = prepare(inputs)
    from concourse.bass_utils import run_bass_kernel_spmd

    all_results = []
    for in_maps in launches:
        res = run_bass_kernel_spmd(nc, in_maps, core_ids=list(range(N_CORES)))
        all_results.append(res.results)
    return combine(all_results)


# revision 22
# speedup vs baseline: 1.6856x; 1.0468x over previous
"""MoE top-k routing + grouped down-proj GEMM + reduce-scatter for trn2 (8 cores).

Problem: intermediate_states [4, 2048, 1024] f16 (rank-sharded expanded-token
activations), w [4, 8, 1024, 2048] f16 (rank-sharded per-expert down-proj),
router_logits [1024, 8] f32, topk=2.  Output [4, 256, 2048] f16.

Strategy: the sparse compute is, per expanded token tk routed to expert e(tk):
y_part[tk] = gate(tk) * (x_full[tk] @ w_full[e(tk)]) with x_full [TK, 4096]
(rank dim folded into the contraction) and w_full[e] [4096, 2048].

Work is decomposed into jobs of (one 128-token tile of one expert) x (one
K-half of 2048).  Each core runs JOBS=5 such jobs: jobs 0-2 share W-slice A,
jobs 3-4 share W-slice B, where a slice is one (expert, K-half) [2048, 2048]
f16 weight block.  The host packs the (expert, K-half) groups (each of
ceil(c_e/128) jobs) into the 8 cores' A/B slots; with sum(ceil(c_e/128)) <= 23
and all c_e <= 384 this always fits, giving near-perfectly balanced PE load
(5 * 16 * 4 matmuls of [128,128]@[128,512] per core).  Each job accumulates in
fp32 PSUM over 16 K-subtiles and applies its fp32 routing gate as a
per-partition scale at PSUM eviction.  No collective is needed: the host sums
each final token's 4 partial rows (topk=2 experts x 2 K-halves).  Routing /
top-k / softmax runs on host in numpy (1024x8 logits - negligible).

Fallback: if some expert has more than 384 tokens the job packing doesn't fit;
an expert-per-core kernel (full K=4096, capacity padded to 128) is used
instead, launched as many times as needed.
"""

import numpy as np

R, T_TOK, TOPK, E = 4, 1024, 2, 8
I_PR, H = 1024, 2048
K = R * I_PR            # 4096 contraction
P = 128
NF = 512                # matmul free-dim (one PSUM bank of fp32)
NH = H // NF            # 4
N_CORES = 8

# job mode
KH = K // 2             # 2048 per K-half
KS2 = KH // P           # 16 k-subtiles per job
JOBS = 5                # jobs per core: 0-2 -> W slice A, 3-4 -> W slice B
SLOT_OF_JOB = (0, 0, 0, 1, 1)
SLOT_CAP = (3, 2)

# fallback (expert-per-core) mode
KSUB = K // P           # 32
CAP_FB = 384            # token capacity per launch in fallback mode

_prog_cache: dict[str, object] = {}


def _new_bacc():
    from concourse import bacc

    return bacc.Bacc(
        "TRN2",
        target_bir_lowering=False,
        debug=False,
        num_devices=N_CORES,
    )


def _build_program_jobs():
    import concourse.mybir as mybir
    import concourse.tile as tile

    f16 = mybir.dt.float16
    f32 = mybir.dt.float32

    nc = _new_bacc()
    # xj[j, p, ks*P + m] = x value of job-j token m at K-row ks*P + p of the
    # job's K-half: exactly the SBUF stationary layout, so loading is a plain
    # 2D DMA per job.
    xj = nc.declare_dram_parameter("xj", [JOBS, P, KS2 * P], f16, isOutput=False)
    wh = nc.declare_dram_parameter("wh", [2, KS2, P, H], f16, isOutput=False)
    gs = nc.declare_dram_parameter("gs", [P, JOBS], f32, isOutput=False)
    ho = nc.declare_dram_parameter("ho", [JOBS, P, H], f16, isOutput=True)

    with tile.TileContext(nc) as tc:
        with tc.tile_pool(name="sb", bufs=1) as sb, \
             tc.tile_pool(name="ps", bufs=2, space="PSUM") as psp:
            # DMA issue order feeds the PE's actual consumption order:
            # stationary x tiles + gates first (2.6MB), then W slice A
            # (jobs 0-2), then W slice B (jobs 3-4).
            # DMA emission order matters twice over: per-DMA issue on the
            # sync queue is ~0.65us regardless of size (so batch small
            # transfers), and the 8 HWDGE queues drain the first wave of DMAs
            # at a fair share of HBM bandwidth (so the first wave must be
            # small and contain the first-needed W chunk).  x tiles are
            # loaded in two halves to keep the first wave lean.
            xt = [sb.tile([P, KS2 * P], f16, name=f"x{j}", tag=f"x{j}", bufs=1)
                  for j in range(JOBS)]
            wt = [[sb.tile([P, H], f16, name=f"w{s}_{ks}", tag=f"w{s}_{ks}",
                           bufs=1) for ks in range(KS2)] for s in range(2)]
            g_raw = sb.tile([P, JOBS], f32, name="g_raw", tag="g_raw", bufs=1)

            HXB = KS2 * P // 2  # half of an x tile's free dim

            def dma_x(j, half):
                sl = slice(half * HXB, (half + 1) * HXB)
                nc.sync.dma_start(xt[j][:, sl], xj[j, :, sl])

            def dma_w(s, ks):
                nc.sync.dma_start(wt[s][ks][:], wh[s, ks, :, :])

            dma_w(0, 0)
            dma_w(0, 1)
            dma_x(0, 0)
            dma_x(1, 0)
            nc.sync.dma_start(g_raw[:], gs[:, :])
            dma_w(0, 2)
            dma_w(0, 3)
            dma_x(0, 1)
            dma_x(1, 1)
            dma_w(0, 4)
            dma_w(0, 5)
            dma_x(2, 0)
            dma_w(0, 6)
            dma_x(2, 1)
            dma_w(0, 7)
            dma_x(3, 0)
            dma_w(0, 8)
            dma_x(3, 1)
            dma_w(0, 9)
            dma_x(4, 0)
            dma_w(0, 10)
            dma_x(4, 1)
            for ks in range(11, KS2):
                dma_w(0, ks)
            for ks in range(KS2):
                dma_w(1, ks)

            # gates: a scalar-engine copy of g_raw; evictions read the copy
            # so their gate dependency is ACT-engine-local and each eviction
            # activation carries only the PE wait (hardware allows a single
            # sync-wait per instruction; bacc splits the rest).
            g2 = sb.tile([P, JOBS], f32, name="g2", tag="g2", bufs=1)
            nc.scalar.copy(g2[:], g_raw[:])

            ps_tiles = {}

            def open_job(j):
                ps_tiles[j] = psp.tile([P, H], f32, name=f"ps{j}", tag="ps",
                                       bufs=2)

            def mm(j, ks):
                s = SLOT_OF_JOB[j]
                lhs = xt[j][:, ks * P:(ks + 1) * P]  # stationary [K=128, M=128]
                for h in range(NH):
                    nc.tensor.matmul(
                        ps_tiles[j][:, h * NF:(h + 1) * NF],
                        lhsT=lhs,
                        rhs=wt[s][ks][:, h * NF:(h + 1) * NF],
                        start=(ks == 0),
                        stop=(ks == KS2 - 1),
                    )

            def evict(j):
                o_t = sb.tile([P, H], f16, name=f"o{j}", tag="o", bufs=JOBS)
                # fp32 gate applied exactly: out = f16(psum_f32 * gate_f32)
                nc.scalar.activation(
                    o_t[:],
                    ps_tiles[j][:],
                    mybir.ActivationFunctionType.Copy,
                    scale=g2[:, j:j + 1],
                )
                nc.sync.dma_start(ho[j, :, :], o_t[:])

            # jobs 0+1 interleaved per chunk (8 matmuls per arriving W chunk
            # keeps the PE ahead of the DMA stream), then job 2 on the
            # now-resident slice A, then jobs 3+4 interleaved on slice B.
            for j in (0, 1):
                open_job(j)
            # HAM warmup: ~40 matmuls on a zeroed tile while the first DMAs
            # stream in.  The PE is otherwise idle here and would run its
            # first ~3.4us of real matmuls at 1.2GHz; garbage results go to
            # job 0's PSUM tile, which the first real matmul (start=True)
            # clears anyway.
            warm_in = sb.tile([P, NF], f16, name="warm_in", tag="warm", bufs=1)
            nc.vector.memset(warm_in[:], 0.0)
            NWARM = 40
            for i in range(NWARM):
                nc.tensor.matmul(
                    ps_tiles[0][:, :NF],
                    lhsT=warm_in[:, :P],
                    rhs=warm_in[:],
                    start=(i == 0),
                    stop=(i == NWARM - 1),
                )
            for ks in range(KS2):
                for j in (0, 1):
                    mm(j, ks)
            for j in (0, 1):
                evict(j)
            open_job(2)
            for ks in range(KS2):
                mm(2, ks)
            evict(2)
            for j in (3, 4):
                open_job(j)
            for ks in range(KS2):
                for j in (3, 4):
                    mm(j, ks)
            for j in (3, 4):
                evict(j)
    nc.finalize()
    return nc


def _build_program_fallback(cap: int):
    import concourse.mybir as mybir
    import concourse.tile as tile

    f16 = mybir.dt.float16
    f32 = mybir.dt.float32
    ntok = cap // P

    nc = _new_bacc()
    xT = nc.declare_dram_parameter("xT", [KSUB, P, cap], f16, isOutput=False)
    wk = nc.declare_dram_parameter("wk", [KSUB, P, H], f16, isOutput=False)
    gs = nc.declare_dram_parameter("gs", [P, ntok], f32, isOutput=False)
    ho = nc.declare_dram_parameter("ho", [ntok, P, H], f16, isOutput=True)

    with tile.TileContext(nc) as tc:
        with tc.tile_pool(name="sb", bufs=1) as sb, \
             tc.tile_pool(name="ps", bufs=2, space="PSUM") as psp:
            xt, wt = [], []
            for k in range(KSUB):
                x_t = sb.tile([P, cap], f16, name=f"x{k}", tag=f"x{k}", bufs=1)
                nc.sync.dma_start(x_t[:], xT[k, :, :])
                w_t = sb.tile([P, H], f16, name=f"w{k}", tag=f"w{k}", bufs=1)
                nc.sync.dma_start(w_t[:], wk[k, :, :])
                xt.append(x_t)
                wt.append(w_t)
            g_raw = sb.tile([P, ntok], f32, name="g_raw", tag="g_raw", bufs=1)
            nc.sync.dma_start(g_raw[:], gs[:, :])
            g2 = sb.tile([P, ntok], f32, name="g2", tag="g2", bufs=1)
            nc.scalar.copy(g2[:], g_raw[:])

            for t in range(ntok):
                ps = psp.tile([P, H], f32, name=f"ps{t}", tag="ps", bufs=2)
                for k in range(KSUB):
                    lhs = xt[k][:, t * P:(t + 1) * P]
                    for h in range(NH):
                        nc.tensor.matmul(
                            ps[:, h * NF:(h + 1) * NF],
                            lhsT=lhs,
                            rhs=wt[k][:, h * NF:(h + 1) * NF],
                            start=(k == 0),
                            stop=(k == KSUB - 1),
                        )
                o_t = sb.tile([P, H], f16, name=f"o{t}", tag="o", bufs=ntok)
                nc.scalar.activation(
                    o_t[:],
                    ps[:],
                    mybir.ActivationFunctionType.Copy,
                    scale=g2[:, t:t + 1],
                )
                nc.sync.dma_start(ho[t, :, :], o_t[:])
    nc.finalize()
    return nc


def _get_program(key):
    if key not in _prog_cache:
        if key == "jobs":
            _prog_cache[key] = _build_program_jobs()
        else:
            _prog_cache[key] = _build_program_fallback(int(key.split(":")[1]))
    return _prog_cache[key]


def _route(logits, topk):
    """numpy replica of jax.lax.top_k + softmax over selected logits."""
    idx = np.argsort(-logits, axis=-1, kind="stable")[:, :topk]      # [T, topk]
    vals = np.take_along_axis(logits, idx, axis=-1)
    mx = vals.max(-1, keepdims=True)
    gate = np.exp(vals - mx)
    gate = gate / gate.sum(-1, keepdims=True)                        # f32
    return idx, gate


def _pack_groups(tiles_per_expert):
    """Assign (expert, khalf) groups to (core, slot).  Returns
    {(e, kh): (core, slot)} or None if infeasible."""
    groups = []
    for e, ntile in enumerate(tiles_per_expert):
        if ntile == 0:
            continue
        for kh in range(2):
            groups.append((ntile, e, kh))
    groups.sort(reverse=True)
    slots = []  # (capacity, core, slot)
    for c in range(N_CORES):
        slots.append([SLOT_CAP[0], c, 0])
        slots.append([SLOT_CAP[1], c, 1])
    # place largest groups first into the fullest-fitting free slot
    assign = {}
    used = [False] * len(slots)
    for ntile, e, kh in groups:
        best = None
        for i, (cap_s, c, s) in enumerate(slots):
            if used[i] or cap_s < ntile:
                continue
            if best is None or cap_s < slots[best][0]:
                best = i
        if best is None:
            return None
        used[best] = True
        assign[(e, kh)] = (slots[best][1], slots[best][2])
    return assign


def prepare(inputs):
    """Host routing + per-core input construction.

    Returns (nc, launches, combine): launches is a list of per-launch in_maps
    (one dict per core); combine(list_of_per_launch_results) -> final output.
    """
    x = np.asarray(inputs["intermediate_states"])          # [R, TK, I_PR] f16
    w = np.asarray(inputs["w"])                            # [R, E, I_PR, H] f16
    logits = np.asarray(inputs["router_logits"]).astype(np.float32)  # [T, E]
    topk = int(np.asarray(inputs["topk"]))

    T, E_ = logits.shape
    TK = T * topk
    assert x.shape == (R, TK, I_PR) and w.shape == (R, E_, I_PR, H) and E_ == E

    idx, gate = _route(logits, topk)
    flat_e = idx.reshape(-1)                               # expert of tk
    counts = np.bincount(flat_e, minlength=E)
    starts = np.zeros(E + 1, np.int64)
    starts[1:] = np.cumsum(counts)
    order = np.argsort(flat_e, kind="stable")              # tks sorted by expert
    g_flat = gate.reshape(TK)
    xf = np.ascontiguousarray(x.transpose(1, 0, 2)).reshape(TK, K)  # [TK, 4096]

    tiles_per_expert = [-(-int(c) // P) for c in counts]
    assign = _pack_groups(tiles_per_expert)
    if assign is not None:
        return _prepare_jobs(w, xf, g_flat, order, starts, counts,
                             tiles_per_expert, assign, topk, T)
    return _prepare_fallback(w, xf, g_flat, order, starts, counts, topk, T)


def _prepare_jobs(w, xf, g_flat, order, starts, counts, tiles_per_expert,
                  assign, topk, T):
    TK = T * topk
    nc = _get_program("jobs")

    xj = np.zeros((N_CORES, JOBS, P, KS2, P), np.float16)
    whs = np.zeros((N_CORES, 2, KS2, P, H), np.float16)
    gss = np.zeros((N_CORES, P, JOBS), np.float32)
    # pos[kh][tk] = row index in the assembled h for token tk's kh-half partial
    pos = np.zeros((2, TK), np.int64)

    job_base = {0: 0, 1: SLOT_CAP[0]}
    for (e, kh), (core, slot) in assign.items():
        toks_e = order[starts[e]:starts[e + 1]]            # ascending tks
        c_e = int(counts[e])
        # w slice: K-half kh of expert e -> [2048, 2048]
        wsl = np.ascontiguousarray(w[2 * kh:2 * kh + 2, e].reshape(KH, H))
        whs[core, slot] = wsl.reshape(KS2, P, H)
        for tt in range(tiles_per_expert[e]):
            j = job_base[slot] + tt
            toks = toks_e[tt * P:(tt + 1) * P]
            n = len(toks)
            # stationary layout [P(krow), KS2, P(tok)] from x K-half kh
            xs = xf[toks, kh * KH:(kh + 1) * KH]           # [n, 2048] f16
            xj[core, j, :, :, :n] = xs.reshape(n, KS2, P).transpose(2, 1, 0)
            gss[core, :n, j] = g_flat[toks]
            pos[kh, toks] = (core * JOBS + j) * P + np.arange(n)

    launches = [[{"xj": xj[c].reshape(JOBS, P, KS2 * P), "wh": whs[c],
                  "gs": gss[c]} for c in range(N_CORES)]]

    def combine(all_results):
        res = all_results[0]
        h_all = np.concatenate(
            [res[c]["ho"].reshape(JOBS * P, H) for c in range(N_CORES)], axis=0)
        y = np.zeros((T, H), np.float32)
        for kh in range(2):
            for kk in range(topk):
                y += h_all[pos[kh, kk::topk]].astype(np.float32)
        return y.astype(np.float16).reshape(R, T // R, H)

    return nc, launches, combine


def _prepare_fallback(w, xf, g_flat, order, starts, counts, topk, T):
    TK = T * topk
    cap_needed = -(-max(int(counts.max()), 1) // P) * P
    cap_launch = min(cap_needed, CAP_FB)
    n_launch = -(-cap_needed // cap_launch)
    cap_total = n_launch * cap_launch
    ntok_l = cap_launch // P

    nc = _get_program(f"fb:{cap_launch}")

    flat_starts = starts[:-1]
    pos = np.empty(TK, np.int64)
    for e in range(E):
        toks = order[starts[e]:starts[e + 1]]
        pos[toks] = e * cap_total + np.arange(len(toks))

    launches = []
    for j in range(n_launch):
        in_maps = []
        for e in range(E):
            toks = order[starts[e]:starts[e + 1]][j * cap_launch:(j + 1) * cap_launch]
            c = len(toks)
            xTe = np.zeros((K, cap_launch), np.float16)
            gse = np.zeros((cap_launch,), np.float32)
            if c:
                xTe[:, :c] = xf[toks].T
                gse[:c] = g_flat[toks]
            in_maps.append({
                "xT": np.ascontiguousarray(xTe.reshape(KSUB, P, cap_launch)),
                "wk": np.ascontiguousarray(w[:, e].reshape(K, H)).reshape(KSUB, P, H),
                "gs": np.ascontiguousarray(gse.reshape(ntok_l, P).T),
            })
        launches.append(in_maps)

    def combine(all_results):
        h_all = np.empty((E * cap_total, H), np.float16)
        for j, res in enumerate(all_results):
            for e in range(E):
                h_all[e * cap_total + j * cap_launch:
                      e * cap_total + (j + 1) * cap_launch] = \
                    res[e]["ho"].reshape(cap_launch, H)
        y = h_all[pos[0::topk]].astype(np.float32)
        for kk in range(1, topk):
            y += h_all[pos[kk::topk]].astype(np.float32)
        return y.astype(np.float16).reshape(R, T // R, H)

    return nc, launches, combine


def kernel(**inputs) -> np.ndarray:
    nc, launches, combine = prepare(inputs)
    from concourse.bass_utils import run_bass_kernel_spmd

    all_results = []
    for in_maps in launches:
        res = run_bass_kernel_spmd(nc, in_maps, core_ids=list(range(N_CORES)))
        all_results.append(res.results)
    return combine(all_results)


# revision 25
# speedup vs baseline: 1.7330x; 1.0281x over previous
"""MoE top-k routing + grouped down-proj GEMM + reduce-scatter for trn2 (8 cores).

Problem: intermediate_states [4, 2048, 1024] f16 (rank-sharded expanded-token
activations), w [4, 8, 1024, 2048] f16 (rank-sharded per-expert down-proj),
router_logits [1024, 8] f32, topk=2.  Output [4, 256, 2048] f16.

Strategy: the sparse compute is, per expanded token tk routed to expert e(tk):
y_part[tk] = gate(tk) * (x_full[tk] @ w_full[e(tk)]) with x_full [TK, 4096]
(rank dim folded into the contraction) and w_full[e] [4096, 2048].

Work is decomposed into jobs of (one 128-token tile of one expert) x (one
K-half of 2048).  Each core runs JOBS=5 such jobs: jobs 0-2 share W-slice A,
jobs 3-4 share W-slice B, where a slice is one (expert, K-half) [2048, 2048]
f16 weight block.  The host packs the (expert, K-half) groups (each of
ceil(c_e/128) jobs) into the 8 cores' A/B slots; with sum(ceil(c_e/128)) <= 23
and all c_e <= 384 this always fits, giving near-perfectly balanced PE load
(5 * 16 * 4 matmuls of [128,128]@[128,512] per core).  Each job accumulates in
fp32 PSUM over 16 K-subtiles and applies its fp32 routing gate as a
per-partition scale at PSUM eviction.  No collective is needed: the host sums
each final token's 4 partial rows (topk=2 experts x 2 K-halves).  Routing /
top-k / softmax runs on host in numpy (1024x8 logits - negligible).

Fallback: if some expert has more than 384 tokens the job packing doesn't fit;
an expert-per-core kernel (full K=4096, capacity padded to 128) is used
instead, launched as many times as needed.
"""

import numpy as np

R, T_TOK, TOPK, E = 4, 1024, 2, 8
I_PR, H = 1024, 2048
K = R * I_PR            # 4096 contraction
P = 128
NF = 512                # matmul free-dim (one PSUM bank of fp32)
NH = H // NF            # 4
N_CORES = 8

# job mode
KH = K // 2             # 2048 per K-half
KS2 = KH // P           # 16 k-subtiles per job
JOBS = 5                # jobs per core: 0-2 -> W slice A, 3-4 -> W slice B
SLOT_OF_JOB = (0, 0, 0, 1, 1)
SLOT_CAP = (3, 2)

# fallback (expert-per-core) mode
KSUB = K // P           # 32
CAP_FB = 384            # token capacity per launch in fallback mode

_prog_cache: dict[str, object] = {}


def _new_bacc():
    from concourse import bacc

    return bacc.Bacc(
        "TRN2",
        target_bir_lowering=False,
        debug=False,
        num_devices=N_CORES,
    )


def _build_program_jobs():
    import concourse.mybir as mybir
    import concourse.tile as tile

    f16 = mybir.dt.float16
    f32 = mybir.dt.float32

    nc = _new_bacc()
    # xj[j, p, ks*P + m] = x value of job-j token m at K-row ks*P + p of the
    # job's K-half: exactly the SBUF stationary layout, so loading is a plain
    # 2D DMA per job.
    xj = nc.declare_dram_parameter("xj", [JOBS, P, KS2 * P], f16, isOutput=False)
    wh = nc.declare_dram_parameter("wh", [2, KS2, P, H], f16, isOutput=False)
    gs = nc.declare_dram_parameter("gs", [P, JOBS], f32, isOutput=False)
    ho = nc.declare_dram_parameter("ho", [JOBS, P, H], f16, isOutput=True)

    with tile.TileContext(nc) as tc:
        with tc.tile_pool(name="sb", bufs=1) as sb, \
             tc.tile_pool(name="ps", bufs=2, space="PSUM") as psp:
            # DMA issue order feeds the PE's actual consumption order:
            # stationary x tiles + gates first (2.6MB), then W slice A
            # (jobs 0-2), then W slice B (jobs 3-4).
            # DMA emission order matters twice over: per-DMA issue on the
            # sync queue is ~0.65us regardless of size (so batch small
            # transfers), and the 8 HWDGE queues drain the first wave of DMAs
            # at a fair share of HBM bandwidth (so the first wave must be
            # small and contain the first-needed W chunk).  x tiles are
            # loaded in two halves to keep the first wave lean.
            xt = [sb.tile([P, KS2 * P], f16, name=f"x{j}", tag=f"x{j}", bufs=1)
                  for j in range(JOBS)]
            wt = [[sb.tile([P, H], f16, name=f"w{s}_{ks}", tag=f"w{s}_{ks}",
                           bufs=1) for ks in range(KS2)] for s in range(2)]
            g_raw = sb.tile([P, JOBS], f32, name="g_raw", tag="g_raw", bufs=1)

            HXB = KS2 * P // 2  # half of an x tile's free dim

            def dma_x(j, half):
                sl = slice(half * HXB, (half + 1) * HXB)
                nc.sync.dma_start(xt[j][:, sl], xj[j, :, sl])

            def dma_w(s, ks):
                nc.sync.dma_start(wt[s][ks][:], wh[s, ks, :, :])

            dma_w(0, 0)
            dma_w(0, 1)
            dma_x(0, 0)
            dma_x(1, 0)
            nc.sync.dma_start(g_raw[:], gs[:, :])
            dma_w(0, 2)
            dma_w(0, 3)
            dma_x(0, 1)
            dma_x(1, 1)
            dma_w(0, 4)
            dma_x(2, 0)
            dma_w(0, 5)
            dma_x(2, 1)
            dma_w(0, 6)
            dma_w(0, 7)
            dma_w(0, 8)
            dma_x(3, 0)
            dma_w(0, 9)
            dma_w(0, 10)
            dma_w(0, 11)
            dma_w(0, 12)
            dma_x(3, 1)
            dma_x(4, 0)
            dma_w(0, 13)
            dma_w(0, 14)
            dma_w(0, 15)
            dma_x(4, 1)
            for ks in range(KS2):
                dma_w(1, ks)

            # gates: a scalar-engine copy of g_raw; evictions read the copy
            # so their gate dependency is ACT-engine-local and each eviction
            # activation carries only the PE wait (hardware allows a single
            # sync-wait per instruction; bacc splits the rest).
            g2 = sb.tile([P, JOBS], f32, name="g2", tag="g2", bufs=1)
            nc.scalar.copy(g2[:], g_raw[:])

            ps_tiles = {}
            HH = H // 2

            def open_job(j):
                # two 2-bank PSUM tiles per job: halves recycle to the next
                # job ~1us sooner than a monolithic 4-bank tile would
                ps_tiles[j] = [
                    psp.tile([P, HH], f32, name=f"ps{j}a", tag="ps", bufs=4),
                    psp.tile([P, HH], f32, name=f"ps{j}b", tag="ps", bufs=4),
                ]

            def mm(j, ks):
                s = SLOT_OF_JOB[j]
                lhs = xt[j][:, ks * P:(ks + 1) * P]  # stationary [K=128, M=128]
                for h in range(NH):
                    nc.tensor.matmul(
                        ps_tiles[j][h // 2][:, (h % 2) * NF:(h % 2 + 1) * NF],
                        lhsT=lhs,
                        rhs=wt[s][ks][:, h * NF:(h + 1) * NF],
                        start=(ks == 0),
                        stop=(ks == KS2 - 1),
                    )

            def evict(j):
                o_t = sb.tile([P, H], f16, name=f"o{j}", tag="o", bufs=JOBS)
                # fp32 gate applied exactly: out = f16(psum_f32 * gate_f32);
                # halves pipelined so the output DMA starts after the first
                for half in range(2):
                    nc.scalar.activation(
                        o_t[:, half * HH:(half + 1) * HH],
                        ps_tiles[j][half][:],
                        mybir.ActivationFunctionType.Copy,
                        scale=g2[:, j:j + 1],
                    )
                    nc.sync.dma_start(ho[j, :, half * HH:(half + 1) * HH],
                                      o_t[:, half * HH:(half + 1) * HH])

            # jobs 0+1 interleaved per chunk (8 matmuls per arriving W chunk
            # keeps the PE ahead of the DMA stream), then job 2 on the
            # now-resident slice A, then jobs 3+4 interleaved on slice B.
            for j in (0, 1):
                open_job(j)
            # HAM warmup: ~40 matmuls on a zeroed tile while the first DMAs
            # stream in.  The PE is otherwise idle here and would run its
            # first ~3.4us of real matmuls at 1.2GHz; garbage results go to
            # job 0's PSUM tile, which the first real matmul (start=True)
            # clears anyway.
            warm_in = sb.tile([P, NF], f16, name="warm_in", tag="warm", bufs=1)
            nc.vector.memset(warm_in[:], 0.0)
            NWARM = 10
            for i in range(NWARM):
                nc.tensor.matmul(
                    ps_tiles[0][0][:, :NF],
                    lhsT=warm_in[:, :P],
                    rhs=warm_in[:],
                    start=(i == 0),
                    stop=(i == NWARM - 1),
                )
            for ks in range(KS2):
                for j in (0, 1):
                    mm(j, ks)
            for j in (0, 1):
                evict(j)
            open_job(2)
            for ks in range(KS2):
                mm(2, ks)
            evict(2)
            for j in (3, 4):
                open_job(j)
            for ks in range(KS2):
                for j in (3, 4):
                    mm(j, ks)
            for j in (3, 4):
                evict(j)
    nc.finalize()
    return nc


def _build_program_fallback(cap: int):
    import concourse.mybir as mybir
    import concourse.tile as tile

    f16 = mybir.dt.float16
    f32 = mybir.dt.float32
    ntok = cap // P

    nc = _new_bacc()
    xT = nc.declare_dram_parameter("xT", [KSUB, P, cap], f16, isOutput=False)
    wk = nc.declare_dram_parameter("wk", [KSUB, P, H], f16, isOutput=False)
    gs = nc.declare_dram_parameter("gs", [P, ntok], f32, isOutput=False)
    ho = nc.declare_dram_parameter("ho", [ntok, P, H], f16, isOutput=True)

    with tile.TileContext(nc) as tc:
        with tc.tile_pool(name="sb", bufs=1) as sb, \
             tc.tile_pool(name="ps", bufs=2, space="PSUM") as psp:
            xt, wt = [], []
            for k in range(KSUB):
                x_t = sb.tile([P, cap], f16, name=f"x{k}", tag=f"x{k}", bufs=1)
                nc.sync.dma_start(x_t[:], xT[k, :, :])
                w_t = sb.tile([P, H], f16, name=f"w{k}", tag=f"w{k}", bufs=1)
                nc.sync.dma_start(w_t[:], wk[k, :, :])
                xt.append(x_t)
                wt.append(w_t)
            g_raw = sb.tile([P, ntok], f32, name="g_raw", tag="g_raw", bufs=1)
            nc.sync.dma_start(g_raw[:], gs[:, :])
            g2 = sb.tile([P, ntok], f32, name="g2", tag="g2", bufs=1)
            nc.scalar.copy(g2[:], g_raw[:])

            for t in range(ntok):
                ps = psp.tile([P, H], f32, name=f"ps{t}", tag="ps", bufs=2)
                for k in range(KSUB):
                    lhs = xt[k][:, t * P:(t + 1) * P]
                    for h in range(NH):
                        nc.tensor.matmul(
                            ps[:, h * NF:(h + 1) * NF],
                            lhsT=lhs,
                            rhs=wt[k][:, h * NF:(h + 1) * NF],
                            start=(k == 0),
                            stop=(k == KSUB - 1),
                        )
                o_t = sb.tile([P, H], f16, name=f"o{t}", tag="o", bufs=ntok)
                nc.scalar.activation(
                    o_t[:],
                    ps[:],
                    mybir.ActivationFunctionType.Copy,
                    scale=g2[:, t:t + 1],
                )
                nc.sync.dma_start(ho[t, :, :], o_t[:])
    nc.finalize()
    return nc


def _get_program(key):
    if key not in _prog_cache:
        if key == "jobs":
            _prog_cache[key] = _build_program_jobs()
        else:
            _prog_cache[key] = _build_program_fallback(int(key.split(":")[1]))
    return _prog_cache[key]


def _route(logits, topk):
    """numpy replica of jax.lax.top_k + softmax over selected logits."""
    idx = np.argsort(-logits, axis=-1, kind="stable")[:, :topk]      # [T, topk]
    vals = np.take_along_axis(logits, idx, axis=-1)
    mx = vals.max(-1, keepdims=True)
    gate = np.exp(vals - mx)
    gate = gate / gate.sum(-1, keepdims=True)                        # f32
    return idx, gate


def _pack_groups(tiles_per_expert):
    """Assign (expert, khalf) groups to (core, slot).  Returns
    {(e, kh): (core, slot)} or None if infeasible."""
    groups = []
    for e, ntile in enumerate(tiles_per_expert):
        if ntile == 0:
            continue
        for kh in range(2):
            groups.append((ntile, e, kh))
    groups.sort(reverse=True)
    slots = []  # (capacity, core, slot)
    for c in range(N_CORES):
        slots.append([SLOT_CAP[0], c, 0])
        slots.append([SLOT_CAP[1], c, 1])
    # place largest groups first into the fullest-fitting free slot
    assign = {}
    used = [False] * len(slots)
    for ntile, e, kh in groups:
        best = None
        for i, (cap_s, c, s) in enumerate(slots):
            if used[i] or cap_s < ntile:
                continue
            if best is None or cap_s < slots[best][0]:
                best = i
        if best is None:
            return None
        used[best] = True
        assign[(e, kh)] = (slots[best][1], slots[best][2])
    return assign


def prepare(inputs):
    """Host routing + per-core input construction.

    Returns (nc, launches, combine): launches is a list of per-launch in_maps
    (one dict per core); combine(list_of_per_launch_results) -> final output.
    """
    x = np.asarray(inputs["intermediate_states"])          # [R, TK, I_PR] f16
    w = np.asarray(inputs["w"])                            # [R, E, I_PR, H] f16
    logits = np.asarray(inputs["router_logits"]).astype(np.float32)  # [T, E]
    topk = int(np.asarray(inputs["topk"]))

    T, E_ = logits.shape
    TK = T * topk
    assert x.shape == (R, TK, I_PR) and w.shape == (R, E_, I_PR, H) and E_ == E

    idx, gate = _route(logits, topk)
    flat_e = idx.reshape(-1)                               # expert of tk
    counts = np.bincount(flat_e, minlength=E)
    starts = np.zeros(E + 1, np.int64)
    starts[1:] = np.cumsum(counts)
    order = np.argsort(flat_e, kind="stable")              # tks sorted by expert
    g_flat = gate.reshape(TK)
    xf = np.ascontiguousarray(x.transpose(1, 0, 2)).reshape(TK, K)  # [TK, 4096]

    tiles_per_expert = [-(-int(c) // P) for c in counts]
    assign = _pack_groups(tiles_per_expert)
    if assign is not None:
        return _prepare_jobs(w, xf, g_flat, order, starts, counts,
                             tiles_per_expert, assign, topk, T)
    return _prepare_fallback(w, xf, g_flat, order, starts, counts, topk, T)


def _prepare_jobs(w, xf, g_flat, order, starts, counts, tiles_per_expert,
                  assign, topk, T):
    TK = T * topk
    nc = _get_program("jobs")

    xj = np.zeros((N_CORES, JOBS, P, KS2, P), np.float16)
    whs = np.zeros((N_CORES, 2, KS2, P, H), np.float16)
    gss = np.zeros((N_CORES, P, JOBS), np.float32)
    # pos[kh][tk] = row index in the assembled h for token tk's kh-half partial
    pos = np.zeros((2, TK), np.int64)

    job_base = {0: 0, 1: SLOT_CAP[0]}
    for (e, kh), (core, slot) in assign.items():
        toks_e = order[starts[e]:starts[e + 1]]            # ascending tks
        c_e = int(counts[e])
        # w slice: K-half kh of expert e -> [2048, 2048]
        wsl = np.ascontiguousarray(w[2 * kh:2 * kh + 2, e].reshape(KH, H))
        whs[core, slot] = wsl.reshape(KS2, P, H)
        for tt in range(tiles_per_expert[e]):
            j = job_base[slot] + tt
            toks = toks_e[tt * P:(tt + 1) * P]
            n = len(toks)
            # stationary layout [P(krow), KS2, P(tok)] from x K-half kh
            xs = xf[toks, kh * KH:(kh + 1) * KH]           # [n, 2048] f16
            xj[core, j, :, :, :n] = xs.reshape(n, KS2, P).transpose(2, 1, 0)
            gss[core, :n, j] = g_flat[toks]
            pos[kh, toks] = (core * JOBS + j) * P + np.arange(n)

    launches = [[{"xj": xj[c].reshape(JOBS, P, KS2 * P), "wh": whs[c],
                  "gs": gss[c]} for c in range(N_CORES)]]

    def combine(all_results):
        res = all_results[0]
        h_all = np.concatenate(
            [res[c]["ho"].reshape(JOBS * P, H) for c in range(N_CORES)], axis=0)
        y = np.zeros((T, H), np.float32)
        for kh in range(2):
            for kk in range(topk):
                y += h_all[pos[kh, kk::topk]].astype(np.float32)
        return y.astype(np.float16).reshape(R, T // R, H)

    return nc, launches, combine


def _prepare_fallback(w, xf, g_flat, order, starts, counts, topk, T):
    TK = T * topk
    cap_needed = -(-max(int(counts.max()), 1) // P) * P
    cap_launch = min(cap_needed, CAP_FB)
    n_launch = -(-cap_needed // cap_launch)
    cap_total = n_launch * cap_launch
    ntok_l = cap_launch // P

    nc = _get_program(f"fb:{cap_launch}")

    flat_starts = starts[:-1]
    pos = np.empty(TK, np.int64)
    for e in range(E):
        toks = order[starts[e]:starts[e + 1]]
        pos[toks] = e * cap_total + np.arange(len(toks))

    launches = []
    for j in range(n_launch):
        in_maps = []
        for e in range(E):
            toks = order[starts[e]:starts[e + 1]][j * cap_launch:(j + 1) * cap_launch]
            c = len(toks)
            xTe = np.zeros((K, cap_launch), np.float16)
            gse = np.zeros((cap_launch,), np.float32)
            if c:
                xTe[:, :c] = xf[toks].T
                gse[:c] = g_flat[toks]
            in_maps.append({
                "xT": np.ascontiguousarray(xTe.reshape(KSUB, P, cap_launch)),
                "wk": np.ascontiguousarray(w[:, e].reshape(K, H)).reshape(KSUB, P, H),
                "gs": np.ascontiguousarray(gse.reshape(ntok_l, P).T),
            })
        launches.append(in_maps)

    def combine(all_results):
        h_all = np.empty((E * cap_total, H), np.float16)
        for j, res in enumerate(all_results):
            for e in range(E):
                h_all[e * cap_total + j * cap_launch:
                      e * cap_total + (j + 1) * cap_launch] = \
                    res[e]["ho"].reshape(cap_launch, H)
        y = h_all[pos[0::topk]].astype(np.float32)
        for kk in range(1, topk):
            y += h_all[pos[kk::topk]].astype(np.float32)
        return y.astype(np.float16).reshape(R, T // R, H)

    return nc, launches, combine


def kernel(**inputs) -> np.ndarray:
    nc, launches, combine = prepare(inputs)
    from concourse.bass_utils import run_bass_kernel_spmd

    all_results = []
    for in_maps in launches:
        res = run_bass_kernel_spmd(nc, in_maps, core_ids=list(range(N_CORES)))
        all_results.append(res.results)
    return combine(all_results)
